# revision 1
# baseline (speedup 1.0000x reference)
"""Trainium2 Bass kernel for nn_Attention_57406532878693 (pooling attention).

Math (per (b, h) slice; T=2048, N=128, K2=16):
    x      = hyp[:, b, h*128:(h+1)*128]                    # (T, N)
    m      = x.mean(0)                                     # (N,)
    gx     = tanh(x @ W_w.T + W_b)                         # (T, K2)
    gm     = tanh(Wm_w @ m + Wm_b)                         # (K2,)
    u      = Wh_w[0] * gm                                  # (K2,)
    l      = gx @ u + Wh_b                                 # (T,)
    p      = exp(l)          (no max-sub needed: |l| <= 4.25, tanh-bounded)
    c      = (p @ x) / p.sum()                             # (N,)
    out[b, h*128:(h+1)*128] = c

Sharding: data-parallel over B across 8 cores (4 batches per core).

Device dataflow per (b, h), 16 t-chunks of 128:
  - natural tiles [t=128p, D] DMA'd once (4KB contiguous rows)
  - transpose each [128,128] chunk on PE (is_transpose, f32r); a tiny
    accumulating matmul (rhs=ones) builds T*mean
  - PSUM->SBUF evacuation of xT split between DVE and ACT
  - gate matmul lhsT=xT chunk, rhs=W_w.T (N=16), bias pre-added into PSUM
    via a K=1 matmul with rhs=tile(W_b, 16)
  - tanh on ACT; mean -> gm -> u path via tiny matmuls
  - logits: DVE mul by broadcast u + grouped reduce; exp on ACT with
    accum_out giving per-partition p-sums; Z via ones matmul
  - weighted sum per head QUAD: lhsT = p_quad[:, c:c+128] (M=128 stride-1
    over-read; rows 0/32/64/96 are the four heads), rhs = four adjacent
    heads' natural chunk [128, 512] -> f32r fast path (1 cyc/row needs
    moving dim >= 256); outputs land on partitions 0/32/64/96 (engine
    accesses must be 32-partition aligned)

Performance (8-core SPMD, measured via tc.For_i loop-slope bench because
this container lacks the axon NTFF profiling hook):
  - HBM load of the 32 MiB/core shard alone: ~84 us/iter (~400 GB/s/core)
  - full kernel: 495 us (pair wsum) -> 412 us (quad wsum, fused mean
    accumulation into PSUM-evacuation copies via accum_out, -42% PE
    instruction count) -> 388 us (ub_ps moved from psgm into psC4 bank to
    shorten psgm lifetime) -> 368 us (bufs psa4/psg2/psc2, sm6, xt4)
  - cost-model TimelineSim predicts ~131-140 us; the gap is per-matmul
    HW overhead (~200-400 ns/instr incl. self-loading fp32r LDWEIGHTS),
    which the cost model does not capture. Remaining PE population:
    512 transposes + 512 gate matmuls dominate.
"""

import os
import numpy as np

T, B, D = 2048, 32, 1024
H, N, K2 = 8, 128, 16
NCORES = 8
BL = B // NCORES          # 4 batches per core
TC = T // 128             # 16 t-chunks
QN = 4                    # t-chunks per DMA/nat tile
NQ = TC // QN             # 4 nat tiles per batch

LAST_RESULT = {}          # exec_time_ns etc. for test harness introspection


def _build(nc, tile, mybir, bass, whb_val, repeat=1, loop_n=0):
    f32 = mybir.dt.float32
    f32r = mybir.dt.float32r
    AF = mybir.ActivationFunctionType

    hyp_s = nc.dram_tensor("hyp_s", [T, BL, D], f32, kind="ExternalInput").ap()
    ident_d = nc.dram_tensor("ident", [128, 128], f32, kind="ExternalInput").ap()
    wg_d = nc.dram_tensor("w_gate", [N, K2], f32, kind="ExternalInput").ap()
    wmg_d = nc.dram_tensor("wm_gate", [N, K2], f32, kind="ExternalInput").ap()
    wbr_d = nc.dram_tensor("wb_rep", [1, TC * K2], f32, kind="ExternalInput").ap()
    wmb_d = nc.dram_tensor("wmb_row", [1, K2], f32, kind="ExternalInput").ap()
    whw_d = nc.dram_tensor("whw_row", [1, K2], f32, kind="ExternalInput").ap()
    ones_d = nc.dram_tensor("ones_col", [128, 2], f32, kind="ExternalInput").ap()
    oner_d = nc.dram_tensor("ones_row", [1, 128], f32, kind="ExternalInput").ap()
    out_s = nc.dram_tensor("out_s", [BL, D], f32, kind="ExternalOutput").ap()

    def r(ap):
        return ap.bitcast(f32r)

    with tile.TileContext(nc) as tc:
        from contextlib import ExitStack, nullcontext

        with ExitStack() as ctx:
            psa_b = int(os.environ.get("KB_PSA", "4"))
            psg_b = int(os.environ.get("KB_PSG", "2"))
            mini_b = int(os.environ.get("KB_MINI", "2"))
            nat_b = int(os.environ.get("KB_NAT", "16"))
            xt_b = int(os.environ.get("KB_XT", "4"))
            cpool = ctx.enter_context(tc.tile_pool(name="consts", bufs=1))
            nat_pool = ctx.enter_context(tc.tile_pool(name="nat", bufs=nat_b))
            xt_pool = ctx.enter_context(tc.tile_pool(name="xt", bufs=xt_b))
            sm_pool = ctx.enter_context(tc.tile_pool(name="small", bufs=6))
            out_pool = ctx.enter_context(tc.tile_pool(name="outp", bufs=1))
            psa_pool = ctx.enter_context(
                tc.tile_pool(name="psa", bufs=psa_b, space="PSUM"))
            psg_pool = ctx.enter_context(
                tc.tile_pool(name="psg", bufs=psg_b, space="PSUM"))
            psc_pool = ctx.enter_context(
                tc.tile_pool(name="psc", bufs=mini_b, space="PSUM"))

            ident = cpool.tile([128, 128], f32, tag="ident")
            nc.sync.dma_start(r(ident[:]), r(ident_d))

            QW = 4 * N                     # 512 cols per head-quad

            def load_nat(b):
                tiles = []
                for hq in range(H // 4):
                    row = []
                    for q in range(NQ):
                        t0 = q * QN * 128
                        nt = nat_pool.tile([128, QN * QW], f32, tag="nat")
                        src = hyp_s[t0:t0 + QN * 128, b:b + 1,
                                    hq * QW:(hq + 1) * QW].rearrange(
                            "(c p) one d -> p c (one d)", p=128)
                        nc.sync.dma_start(
                            r(nt[:].rearrange("p (c d) -> p c d", c=QN)),
                            r(src))
                        row.append(nt)
                    tiles.append(row)
                return tiles

            nat0 = load_nat(0) if not loop_n else None

            wg = cpool.tile([N, K2], f32, tag="wg")
            nc.sync.dma_start(wg[:], wg_d)
            wmg = cpool.tile([N, K2], f32, tag="wmg")
            nc.sync.dma_start(wmg[:], wmg_d)
            wbr = cpool.tile([1, TC * K2], f32, tag="wbr")
            nc.sync.dma_start(r(wbr[:]), r(wbr_d))
            wmb = cpool.tile([1, K2], f32, tag="wmb")
            nc.sync.dma_start(wmb[:], wmb_d)
            whw = cpool.tile([1, K2], f32, tag="whw")
            nc.sync.dma_start(whw[:], whw_d)
            ones_c = cpool.tile([128, 2], f32, tag="ones")
            nc.sync.dma_start(r(ones_c[:]), r(ones_d))
            oner = cpool.tile([1, 128], f32, tag="oner")
            nc.sync.dma_start(r(oner[:]), r(oner_d))
            oner2 = cpool.tile([1, 128], f32, tag="oner2")
            nc.sync.dma_start(oner2[:], oner_d)
            whb_c = cpool.tile([128, 1], f32, tag="whb")
            nc.gpsimd.memset(whb_c[:], float(whb_val))

            out_sb = out_pool.tile([97, BL * D // 4], f32, tag="out")

            def do_batch(b, nat):
                def xchunk(c, h):
                    base = (c % QN) * QW + (h % 4) * N
                    return nat[h // 4][c // QN][:, base:base + N]

                for hq in range(H // 4):
                    psC4 = psc_pool.tile([128, 512], f32, tag="psc")
                    p_quad = sm_pool.tile([128, 144], f32, tag="p_quad")
                    pr_quad = sm_pool.tile([128, 97], f32, tag="pr_quad")
                    nc.gpsimd.memset(p_quad[:], 0.0)
                    nc.gpsimd.memset(pr_quad[:], 1.0)
                    z_psgm = None

                    for q in range(4):
                        h = 4 * hq + q
                        xt = xt_pool.tile([128, T], f32, tag="xt")
                        psgm = psg_pool.tile([128, 512], f32, tag="psgm")
                        psG = psgm[:, 0:TC * K2]      # [128, 256]
                        gm_ps = psgm[0:1, 288:288 + K2]
                        ub_ps = psC4[:, 32 * q:32 * q + K2]
                        if q == 3:
                            z_psgm = psgm

                        msum = sm_pool.tile([128, 4], f32, tag="msum")

                        # bias pre-add: psG := ones(128,1) @ wb_rep(1,256)
                        nc.tensor.matmul(psG, r(oner[:]), r(wbr[:]),
                                         start=True, stop=False,
                                         skip_group_check=True)

                        for g in range(4):
                            psa = psa_pool.tile([128, 512], f32, tag="psa")
                            for j in range(4):
                                c = g * 4 + j
                                xc = xchunk(c, h)
                                nc.tensor.transpose(
                                    r(psa[:, j * 128:(j + 1) * 128]), r(xc),
                                    r(ident[:]))
                            dst = xt[:, g * 512:(g + 1) * 512]
                            if g % 2 == 0:
                                nc.vector.tensor_scalar(
                                    dst, psa[:], 1.0, 0.0,
                                    op0=mybir.AluOpType.mult,
                                    op1=mybir.AluOpType.add,
                                    accum_out=msum[:, g:g + 1])
                            else:
                                nc.scalar.activation(
                                    dst, psa[:], AF.Copy, bias=0.0,
                                    scale=1.0, accum_out=msum[:, g:g + 1])
                            for j in range(4):
                                c = g * 4 + j
                                nc.tensor.matmul(
                                    psG[:, c * K2:(c + 1) * K2],
                                    xt[:, c * 128:(c + 1) * 128], wg[:],
                                    start=False, stop=True,
                                    skip_group_check=True)

                        g_sb = sm_pool.tile([128, TC * K2], f32, tag="g_sb")
                        nc.scalar.activation(g_sb[:], psG, AF.Tanh)

                        # m_sb holds T*mean; 1/T is folded into wm_gate
                        m_sb = sm_pool.tile([128, 1], f32, tag="m_sb")
                        nc.vector.tensor_reduce(
                            m_sb[:], msum[:],
                            axis=mybir.AxisListType.X, op=mybir.AluOpType.add)

                        nc.tensor.matmul(gm_ps, m_sb[:], wmg[:],
                                         start=True, stop=False,
                                         skip_group_check=True)
                        nc.tensor.matmul(gm_ps, oner2[:, 0:1], wmb[:],
                                         start=False, stop=True,
                                         skip_group_check=True)
                        gm_sb = sm_pool.tile([1, K2], f32, tag="gm_sb")
                        nc.scalar.activation(gm_sb[:], gm_ps, AF.Tanh)
                        u_sb = sm_pool.tile([1, K2], f32, tag="u_sb")
                        nc.vector.tensor_mul(u_sb[:], gm_sb[:], whw[:])
                        nc.tensor.matmul(ub_ps, oner2[:], u_sb[:],
                                         start=True, stop=True,
                                         skip_group_check=True)

                        lp_sb = sm_pool.tile([128, TC * K2], f32, tag="lp_sb")
                        nc.vector.tensor_mul(
                            lp_sb[:].rearrange("p (c k) -> p c k", k=K2),
                            g_sb[:].rearrange("p (c k) -> p c k", k=K2),
                            ub_ps.unsqueeze(1).broadcast_to([128, TC, K2]))
                        l_sb = sm_pool.tile([128, TC], f32, tag="l_sb")
                        nc.vector.tensor_reduce(
                            l_sb[:],
                            lp_sb[:].rearrange("p (c k) -> p c k", k=K2),
                            axis=mybir.AxisListType.X, op=mybir.AluOpType.add)

                        with nc.allow_low_precision(
                                reason="f32r accum is fp32-width"):
                            nc.scalar.activation(
                                r(p_quad[:, 32 * q:32 * q + TC]), l_sb[:],
                                AF.Exp, bias=whb_c[:], scale=1.0,
                                accum_out=r(pr_quad[:, 32 * q:32 * q + 1]))

                    z_ps = z_psgm[0:97, 352:354]
                    nc.tensor.matmul(z_ps, r(pr_quad[:]), r(ones_c[:]),
                                     start=True, stop=True,
                                     skip_group_check=True)
                    zi_sb = sm_pool.tile([97, 1], f32, tag="zi_sb")
                    nc.vector.reciprocal(zi_sb[:], z_ps[0:97, 0:1])

                    for c in range(TC):
                        lhs4 = p_quad[:, c:c + 128]
                        rhs4 = nat[hq][c // QN][:, (c % QN) * QW:
                                                (c % QN + 1) * QW]
                        nc.tensor.matmul(psC4[:], r(lhs4), r(rhs4),
                                         start=(c == 0), stop=(c == TC - 1),
                                         skip_group_check=True)

                    for q in range(4):
                        col = b * (D // 4) + hq * N
                        nc.scalar.activation(
                            out_sb[32 * q:32 * q + 1, col:col + N],
                            psC4[32 * q:32 * q + 1, q * N:(q + 1) * N],
                            AF.Copy, bias=0.0,
                            scale=zi_sb[32 * q:32 * q + 1, 0:1])

            if loop_n:
                with tc.For_i(0, loop_n, 1):
                    for b in range(BL):
                        do_batch(b, load_nat(b))
            else:
                sched = [bb for _ in range(repeat) for bb in range(BL)]
                for it, b in enumerate(sched):
                    do_batch(b, nat0 if it == 0 else load_nat(b))

            for q in range(4):
                nc.sync.dma_start(
                    out_s.rearrange("b (j q n) -> q b j n", q=4, n=N)[q:q + 1],
                    out_sb[32 * q:32 * q + 1, :].rearrange(
                        "one (b j n) -> one b j n", j=H // 4, n=N))
    return nc


def kernel(**inputs):
    import concourse.bass as bass
    import concourse.bacc as bacc
    import concourse.tile as tile
    import concourse.mybir as mybir
    from concourse import bass_utils

    hyp = np.ascontiguousarray(np.asarray(inputs["hyp"], dtype=np.float32))
    W_w = np.asarray(inputs["W_w"], dtype=np.float32)      # (K2, N)
    W_b = np.asarray(inputs["W_b"], dtype=np.float32)      # (K2,)
    Wm_w = np.asarray(inputs["Wm_w"], dtype=np.float32)    # (K2, N)
    Wm_b = np.asarray(inputs["Wm_b"], dtype=np.float32)    # (K2,)
    Wh_w = np.asarray(inputs["Wh_w"], dtype=np.float32)    # (1, K2)
    Wh_b = np.asarray(inputs["Wh_b"], dtype=np.float32)    # (1,)

    nc = bacc.Bacc("TRN2", target_bir_lowering=False, debug=False)
    _build(nc, tile, mybir, bass, float(Wh_b.reshape(-1)[0]))
    nc.compile()

    consts = {
        "ident": np.eye(128, dtype=np.float32),
        "w_gate": np.ascontiguousarray(W_w.T),             # (N, K2)
        "wm_gate": np.ascontiguousarray(Wm_w.T) / T,       # (N, K2), 1/T folded
        "wb_rep": np.tile(W_b, TC)[None, :].astype(np.float32),
        "wmb_row": Wm_b[None, :].astype(np.float32),
        "whw_row": Wh_w.reshape(1, K2).astype(np.float32),
        "ones_col": np.ones((128, 2), np.float32),
        "ones_row": np.ones((1, 128), np.float32),
    }
    in_maps = []
    for j in range(NCORES):
        m = {"hyp_s": np.ascontiguousarray(hyp[:, j * BL:(j + 1) * BL, :])}
        m.update(consts)
        in_maps.append(m)

    trace = os.environ.get("BASS_KERNEL_TRACE", "0") == "1"
    res = bass_utils.run_bass_kernel_spmd(
        nc, in_maps, core_ids=list(range(NCORES)), trace=trace)

    LAST_RESULT.clear()
    LAST_RESULT["exec_time_ns"] = res.exec_time_ns
    LAST_RESULT["trace"] = (res.instructions_and_trace[1]
                            if res.instructions_and_trace else None)
    LAST_RESULT["profile_json"] = res.profile_json

    out = np.concatenate([res.results[j]["out_s"] for j in range(NCORES)],
                         axis=0)
    return out.astype(np.float32)

